# revision 80
# baseline (speedup 1.0000x reference)
"""Multi-head causal self-attention with RoPE on 8 Trainium2 NeuronCores.

Sharding: 16 heads -> 8 cores (2 heads/core, head/tensor parallel).
Wq/Wk/Wv column-sharded (per-head-group rows of W), Wo row-sharded.
Each core computes a full (S, D) partial of the output projection in fp16;
the host sums the 8 partials (the row-parallel reduce).

v3 layout notes (cost-model driven, on top of v2):
 - q-tiles processed in DESCENDING order so the tail tile is the smallest
   (qt=0: 4 chunks) -> short serial drain at the end.
 - RoPE pair layout regrouped per 32-partition quadrant ([e0..15 o0..15]
   per 32 rows) so the even/odd swap is a single DVE stream_shuffle per
   tile instead of 4 partition-swap DMAs (saves ~630ns HWDGE + ~600ns SEQ
   per DMA); the sin-multiply runs on the otherwise-idle GPSIMD.
 - exp work split Act/DVE by greedy cost balance (~53/47) instead of 75/25;
   DVE chunks use the Schraudolph int16 bit-trick, Act chunks exact exp.
 - psum drains (vt/v1/yrow) greedy-balanced across Act/DVE as well.
 - weights stored pre-swizzled in DRAM ([p, c*m] rows) for full-bandwidth
   2KB-row DMAs; startup DMAs split so the first projection matmul starts
   after ~0.4MB of traffic instead of ~4.5MB.
"""

import sys

for _p in ("/opt/trn_rl_repo", "/root/.axon_site/_ro/trn_rl_repo"):
    if _p not in sys.path:
        sys.path.insert(0, _p)

import numpy as np

S_FULL = 4096
D = 1024
NH = 16
DK = 64
P = 128
QT = 512  # q tile (free dim of score tiles)
KC = 128  # k chunk (partition dim of score tiles)
DC = D // P  # 8 contraction chunks for the projections
THETA = 10000.0
N_CORES = 8

# Schraudolph exp constants for fp16 bit pattern: round(s*A + B) ~ fp16(exp(s/8))
LOG2E = 1.4426950408889634
EXP_A = (1 << 10) * LOG2E * 0.125
EXP_B = 15.0 * (1 << 10) - 58.9

# engine-cost constants (ns) used for greedy Act/DVE balancing at build time
ACT_NS = 0.8333
DVE_NS = 1.0417
ACT_OV = 185.0
DVE_OV = 125.0

_BUILD_CACHE: dict = {}


class EngBalance:
    """Greedy Act/DVE load balancer for psum-reading elementwise ops.

    act0 pre-loads the Act side to account for work outside the balancer's
    view (SEQ overheads, exact-exp affinity) observed in the trace.
    """

    def __init__(self, act0=0.0):
        self.act = act0
        self.dve = 0.0

    def pick(self, cols, bias_act=0.0):
        act_c = cols * ACT_NS + ACT_OV
        dve_c = cols * DVE_NS + DVE_OV
        if self.act + act_c + bias_act <= self.dve + dve_c:
            self.act += act_c
            return "act"
        self.dve += dve_c
        return "dve"


def build(S: int = S_FULL, reps: int = 1):
    """Build the per-core Bass program (same program for all cores)."""
    key = (S, reps)
    if key in _BUILD_CACHE:
        return _BUILD_CACHE[key]

    import concourse.bacc as bacc
    import concourse.tile as tile
    from concourse import mybir

    f32 = mybir.dt.float32
    f16 = mybir.dt.float16
    i16 = mybir.dt.int16
    Alu = mybir.AluOpType
    Act = mybir.ActivationFunctionType

    NQ = S // QT
    NK = S // KC
    DIAG = QT // KC  # k-chunks per q-tile on the diagonal (4)
    SCALE = float(DK) ** -0.5
    # big/small interleave: every small-tile iteration carries a big attnV
    # stream (and vice versa), the exp pipeline stays ~balanced against PE,
    # and the tail tile (qt=3) is big enough to self-overlap its chains.
    ORDER = []
    lo, hi = 0, NQ - 1
    while lo <= hi:
        ORDER.append(hi)
        if lo < hi:
            ORDER.append(lo)
        hi -= 1
        lo += 1
    ESB_N = max(DIAG * ORDER[p] + DIAG for p in range(1, NQ, 2))
    SHUF_MASK = list(range(16, 32)) + list(range(0, 16))  # swap 16-halves

    nc = bacc.Bacc(
        "TRN2", target_bir_lowering=False, debug=False, num_devices=N_CORES
    )
    xT = nc.dram_tensor("xT", [D, S], f16, kind="ExternalInput")
    # weights pre-swizzled on host to [p, c, w(q|k|v), m] rows so startup is
    # 2 DMA instructions, not 6 (each DMA pays ~630ns on the shared HWDGE)
    wqkvR = nc.dram_tensor("wqkvR", [P, DC * 3 * P], f16, kind="ExternalInput")
    csd = nc.dram_tensor("csd", [P, 2 * S], f16, kind="ExternalInput")
    # wo | mask | ident concatenated: one late-constants DMA
    cstd = nc.dram_tensor("cstd", [P, D + DIAG * QT + P], f16, kind="ExternalInput")
    yT = nc.dram_tensor("yT", [D, S], f16, kind="ExternalOutput")

    bal = EngBalance(act0=0.0)

    with tile.TileContext(nc) as tc:
        with (
            tc.tile_pool(name="const", bufs=1) as cp,
            tc.tile_pool(name="persist", bufs=1) as pp,
        ):
            # ---- constants ----
            wqkv_sb = cp.tile([P, DC, 3, P], f16, tag="wqkv")
            cs_sb = cp.tile([P, 2, S], f16, tag="cs")
            cst_sb = cp.tile([P, D + DIAG * QT + P], f16, tag="cst")
            wo_sb = cst_sb[:, 0:D]
            mask_sb = cst_sb[:, D : D + DIAG * QT].rearrange(
                "p (j q) -> p j q", j=DIAG
            )
            id_sb = cst_sb[:, D + DIAG * QT :]

            # ---- persistent activations ----
            qT_sb = pp.tile([P, S], f16, tag="qT")
            kT_sb = pp.tile([P, S], f16, tag="kT")
            v1a = pp.tile([P, NK, 65], f16, tag="v1a")  # head 0: [v, ones]
            v1b = pp.tile([P, NK, 65], f16, tag="v1b")  # head 1
            # es double-buffered by position parity; even positions are the
            # larger tiles (qt = 7,5,3,1 -> up to NK chunks)
            es_a = pp.tile([P, NK, 2, QT], f16, tag="esa")
            es_b = pp.tile([P, ESB_N, 2, QT], f16, tag="esb")
            attnT = pp.tile([P, 2, QT], f16, tag="attnT")  # parity-buffered

            def es_for(pos):
                return es_a if pos % 2 == 0 else es_b

            def emit_exp(es_f16, es_i16, ps_ap, cols, force_act=False):
                """ps_ap: psum fp32 source; es_*: fp16/int16 SBUF views."""
                if force_act:
                    bal.act += cols * ACT_NS + ACT_OV
                    eng = "act"
                else:
                    eng = bal.pick(cols)
                if eng == "act":
                    nc.scalar.activation(es_f16, ps_ap, Act.Exp, scale=SCALE)
                else:
                    nc.vector.tensor_scalar(
                        out=es_i16,
                        in0=ps_ap,
                        scalar1=float(EXP_A),
                        scalar2=float(EXP_B),
                        op0=Alu.mult,
                        op1=Alu.add,
                    )

            def score_unit(pool, pos, qt, kc):
                q0 = qt * QT
                es_all = es_for(pos)
                j = kc - DIAG * qt  # >=0 on the diagonal
                qoff = j * KC if j >= 0 else 0
                ksl = slice(kc * KC, (kc + 1) * KC)
                ps = pool.tile([P, 2, QT], f32, tag="sc")
                diag = j >= 0
                if diag:
                    # causal mask folded in as a -1e4 bias via an
                    # identity matmul into the psum (per head)
                    for h in range(2):
                        nc.tensor.matmul(
                            ps[:, h, qoff : qoff + KC],
                            id_sb,
                            mask_sb[:, j, qoff : qoff + KC],
                            start=True, stop=False,
                        )
                nc.tensor.matmul(
                    ps[:, 0, qoff:QT],
                    kT_sb[0:64, ksl],
                    qT_sb[0:64, q0 + qoff : q0 + QT],
                    start=not diag, stop=True, tile_position=(0, 0),
                )
                nc.tensor.matmul(
                    ps[:, 1, qoff:QT],
                    kT_sb[64:128, ksl],
                    qT_sb[64:128, q0 + qoff : q0 + QT],
                    start=not diag, stop=True, tile_position=(64, 0),
                )
                if j >= 1:
                    # narrow per-head exp (saves engine cols on the diag);
                    # diag chunks carry the big probs -> exact exp on Act
                    for h in range(2):
                        emit_exp(
                            es_all[:, kc, h, qoff:QT],
                            es_all.bitcast(i16)[:, kc, h, qoff:QT],
                            ps[:, h, qoff:QT],
                            QT - qoff,
                            force_act=True,
                        )
                else:
                    emit_exp(
                        es_all[:, kc, :, :].rearrange("p h q -> p (h q)"),
                        es_all.bitcast(i16)[:, kc, :, :].rearrange(
                            "p h q -> p (h q)"
                        ),
                        ps.rearrange("p h q -> p (h q)"),
                        2 * QT,
                        force_act=(j == 0),
                    )

            # ---- phase A: projections + RoPE + v-transposes, per 512-col
            # chunk; the first phase-E position's (qt = NQ-1) scores + exp
            # are FUSED into this phase: its q/k tile ropes first, and each
            # roped k-tile's 4 score chunks emit one tile-section later,
            # interspersed in the projection c-loop. Their psums rotate in a
            # dedicated 2-buf pool that closes with phase A. ----
            TORD = list(range(NQ))  # tile process order
            with (
                tc.tile_pool(name="xc", bufs=4) as xcp,
                tc.tile_pool(name="rope", bufs=2) as rp,
                tc.tile_pool(name="proj_ps", bufs=2, space="PSUM") as pps,
                tc.tile_pool(name="tp_ps", bufs=2, space="PSUM") as tpp,
            ):
                def fetch_chunk(nt):
                    sl = slice(nt * QT, (nt + 1) * QT)
                    xc = xcp.tile([P, DC, QT], f16, tag="xc")
                    xv = xT[:, sl].rearrange("(c p) q -> p c q", p=P)
                    if nt == TORD[0]:
                        # split so the first projection matmuls start sooner
                        nc.scalar.dma_start(out=xc[:, 0:1, :], in_=xv[:, 0:1, :])
                        nc.scalar.dma_start(out=xc[:, 1:3, :], in_=xv[:, 1:3, :])
                        nc.scalar.dma_start(out=xc[:, 3:DC, :], in_=xv[:, 3:DC, :])
                    else:
                        nc.scalar.dma_start(out=xc, in_=xv)
                    return xc

                # the scalar(Act) DMA queue carries ONLY x chunks; everything
                # else rides the sync(SP) queue ordered by first-use time
                wv3 = wqkvR[:, :].rearrange("p (c w m) -> p c w m", c=DC, w=3)
                csv = csd[:, :].rearrange("p (t s) -> p t s", t=2)
                # c0 chunk of all three weights first (tile0's c-loop
                # interleaves q/k/v matmuls, so all three gate the start)
                nc.sync.dma_start(out=wqkv_sb[:, 0:1, :, :], in_=wv3[:, 0:1, :, :])
                prefs = [fetch_chunk(TORD[0]), fetch_chunk(TORD[1]), fetch_chunk(TORD[2])]
                nc.sync.dma_start(out=wqkv_sb[:, 1:DC, :, :], in_=wv3[:, 1:DC, :, :])
                # cos/sin for the first two tiles in process order; the rest
                # streams per-2-tiles inside the loop so the big transfers
                # don't crowd the x chunks off the DMA pool
                nc.sync.dma_start(out=cs_sb[:, :, 0 : 2 * QT], in_=csv[:, :, 0 : 2 * QT])
                # Z-accumulator ones column via GPSIMD memset (no DMA needed)
                nc.gpsimd.memset(v1a[:, :, 64:65], 1.0)
                nc.gpsimd.memset(v1b[:, :, 64:65], 1.0)

                vt_prev = None
                for idx in range(NQ):
                    nt = TORD[idx]
                    sl = slice(nt * QT, (nt + 1) * QT)
                    xc = prefs.pop(0)
                    if idx + 3 < NQ:
                        prefs.append(fetch_chunk(TORD[idx + 3]))
                    if idx in (1, 3, 5):
                        s0 = nt * QT + QT  # next two tiles in ascending order
                        nc.sync.dma_start(
                            out=cs_sb[:, :, s0 : s0 + 2 * QT],
                            in_=csv[:, :, s0 : s0 + 2 * QT],
                        )
                    if idx == 0:
                        # ident (needed by vtrans(t0) at ~10us) + wo/mask
                        nc.sync.dma_start(out=cst_sb, in_=cstd[:, :])
                    psq = pps.tile([P, QT], f32, tag="psq")
                    psk = pps.tile([P, QT], f32, tag="psk")
                    psv = pps.tile([P, QT], f32, tag="psv")
                    for c in range(DC):
                        st, sp = (c == 0), (c == DC - 1)
                        nc.tensor.matmul(psq, wqkv_sb[:, c, 0, :], xc[:, c, :], start=st, stop=sp)
                        nc.tensor.matmul(psk, wqkv_sb[:, c, 1, :], xc[:, c, :], start=st, stop=sp)
                        nc.tensor.matmul(psv, wqkv_sb[:, c, 2, :], xc[:, c, :], start=st, stop=sp)
                    vt = rp.tile([P, QT], f16, tag="vt")
                    nc.scalar.copy(qT_sb[:, sl], psq)
                    nc.scalar.copy(kT_sb[:, sl], psk)
                    nc.scalar.copy(vt, psv)
                    # RoPE on this chunk (in place): the even/odd swap is one
                    # DVE stream_shuffle (pairs are 16 apart within each
                    # 32-partition quadrant); sin-mul runs on GPSIMD
                    for src_sb, tgname in ((qT_sb, "swq"), (kT_sb, "swk")):
                        sw = rp.tile([P, QT], f16, tag=tgname)
                        nc.vector.stream_shuffle(sw, src_sb[:, sl], SHUF_MASK)
                        m1 = rp.tile([P, QT], f16, tag="m1", name="m1")
                        nc.vector.tensor_mul(m1, src_sb[:, sl], cs_sb[:, 0, sl])
                        nc.gpsimd.tensor_mul(sw, sw, cs_sb[:, 1, sl])
                        nc.vector.tensor_add(src_sb[:, sl], m1, sw)

                    # v~ transposes lag one chunk so PE never waits on the
                    # fresh vt copy
                    def vtrans(nt, vt):
                        for h, v1 in ((0, v1a), (1, v1b)):
                            hp = h * 64
                            pst = tpp.tile([P, DIAG, 64], f16, tag="pst", name="pst")
                            for j in range(DIAG):
                                nc.tensor.transpose(
                                    pst[:, j, :],
                                    vt[hp : hp + 64, j * KC : (j + 1) * KC],
                                    id_sb[hp : hp + 64, hp : hp + 64],
                                )
                            nc.scalar.copy(
                                v1[:, DIAG * nt : DIAG * nt + DIAG, 0:64], pst
                            )
                    if vt_prev is not None:
                        vtrans(*vt_prev)
                    vt_prev = (nt, vt)
                if vt_prev is not None:
                    vtrans(*vt_prev)

            # ---- phase E: scores+exp, attnV, outproj, per q tile ----
            with (
                tc.tile_pool(name="sc_ps", bufs=3, space="PSUM") as scp,
                tc.tile_pool(name="att_ps", bufs=2, space="PSUM") as attp,
                tc.tile_pool(name="asb", bufs=4) as asp,
                tc.tile_pool(name="yrow", bufs=3) as yrp,
            ):
                def attn_units(pos, qt):
                    """Closures for attnV matmul steps, norms, and outproj of
                    q-tile qt; interleaved between scores chunks of the next
                    position so the in-order PE queue always has ready work."""
                    es_all = es_for(pos)
                    par = pos % 2
                    q0 = qt * QT
                    state = {}

                    def pa_slot(qcl, h):
                        # per head: [h*66 : h*66+66] = [attn 65 | Z 1]; f16
                        # cols [264:392] of the tile (same bank) are the
                        # PE-transpose scratch. attnV start=True zeroes the
                        # whole bank, so each qcl gets its OWN tile; pool
                        # rotation orders the bank wipe after the previous
                        # qcl's transpose copy.
                        return state[qcl][:, h * 66 : h * 66 + 66]

                    def mk_av(qcl, kc, qc):
                        def f():
                            if kc == 0:
                                if pos >= NQ - 2:
                                    # tail: rotate through the idle 3-deep
                                    # score pool so qcl chains overlap
                                    t = scp.tile(
                                        [P, 2, QT], f32, tag="sc", name="pa"
                                    )
                                    state[qcl] = t[:, 0, 0:196]
                                    state["tp%d" % qcl] = t.bitcast(f16)[
                                        :, 0, 264:392
                                    ]
                                else:
                                    t = attp.tile(
                                        [P, 196], f32, tag="att", name="pa"
                                    )
                                    state[qcl] = t
                                    state["tp%d" % qcl] = t.bitcast(f16)[:, 264:392]
                            nc.tensor.matmul(
                                pa_slot(qcl, 0)[:, 0:65],
                                es_all[:, kc, 0, qcl * KC : (qcl + 1) * KC],
                                v1a[:, kc, :],
                                start=(kc == 0), stop=False,
                            )
                            nc.tensor.matmul(
                                pa_slot(qcl, 1)[:, 0:65],
                                es_all[:, kc, 1, qcl * KC : (qcl + 1) * KC],
                                v1b[:, kc, :],
                                start=False, stop=(kc == qc),
                            )
                        return f

                    def mk_norm(qcl):
                        def f():
                            zrec = asp.tile([P, 2, 1], f32, tag="zrec", name="zrec")
                            asb = asp.tile([P, P], f16, tag="asb", name="asb")
                            zs = state[qcl][:, 0:132].rearrange(
                                "p (h c) -> p h c", h=2
                            )[:, :, 64:65]
                            nc.vector.reciprocal(zrec, zs)
                            bal.dve += 2 * DVE_NS + DVE_OV
                            for h in range(2):
                                if bal.pick(64) == "act":
                                    nc.scalar.activation(
                                        asb[:, h * 64 : (h + 1) * 64],
                                        pa_slot(qcl, h)[:, 0:64],
                                        Act.Copy,
                                        scale=zrec[:, h, :],
                                    )
                                else:
                                    nc.vector.tensor_scalar(
                                        out=asb[:, h * 64 : (h + 1) * 64],
                                        in0=pa_slot(qcl, h)[:, 0:64],
                                        scalar1=zrec[:, h, :],
                                        scalar2=None,
                                        op0=Alu.mult,
                                    )
                            state["asb%d" % qcl] = asb
                        return f

                    def mk_tpose(qcl):
                        def f():
                            asb = state.pop("asb%d" % qcl)
                            tp = state.pop("tp%d" % qcl)
                            nc.tensor.transpose(tp, asb, id_sb)
                            if bal.pick(KC) == "act":
                                nc.scalar.copy(
                                    attnT[:, par, qcl * KC : (qcl + 1) * KC], tp
                                )
                            else:
                                nc.vector.tensor_copy(
                                    attnT[:, par, qcl * KC : (qcl + 1) * KC], tp
                                )
                        return f

                    def mk_po(oc):
                        def f():
                            if oc % 2 == 0:
                                # oc-pair staging tiles, 3-deep rotation, so a
                                # drain never waits on the previous position's
                                # full output DMA
                                state["yr"] = yrp.tile([P, 2, QT], f16, tag="yrow", name="yrow")
                            # outproj psums ride the 3-deep score ring (no
                            # dedicated po bank; its bank buys att bufs=2)
                            po = scp.tile([P, 2, QT], f32, tag="sc", name="po")[:, 0, :]
                            if pos >= NQ - 2:
                                # tail: split by q-subblock so each sub-matmul
                                # only waits on its own qcl's DMA-transpose
                                for qcl in range(DIAG):
                                    qs = slice(qcl * KC, (qcl + 1) * KC)
                                    nc.tensor.matmul(
                                        po[:, qs],
                                        wo_sb[:, oc * P : (oc + 1) * P],
                                        attnT[:, par, qs],
                                        start=True, stop=True,
                                    )
                            else:
                                nc.tensor.matmul(
                                    po, wo_sb[:, oc * P : (oc + 1) * P],
                                    attnT[:, par, :],
                                    start=True, stop=True,
                                )
                            if bal.pick(QT) == "act":
                                nc.scalar.copy(state["yr"][:, oc % 2, :], po)
                            else:
                                nc.vector.tensor_copy(state["yr"][:, oc % 2, :], po)
                            qsl = slice(q0, q0 + QT)
                            yv = yT[:, qsl].rearrange("(c p) q -> p c q", p=P)
                            if oc % 2 == 1:
                                nc.sync.dma_start(
                                    out=yv[:, oc - 1 : oc + 1, :], in_=state["yr"]
                                )
                        return f

                    # weave: each qcl's PE-transpose unit is delayed 2 slots
                    # into the next qcl's stream so it doesn't park the
                    # in-order PE queue while waiting on the asb scales
                    av_units = []
                    pend_tp = None
                    for qcl in range(DIAG):
                        lst = []
                        qc = DIAG * qt + qcl
                        for kc in range(qc + 1):
                            lst.append(mk_av(qcl, kc, qc))
                        lst.append(mk_norm(qcl))
                        if pend_tp is not None:
                            cut = min(2, len(lst) - 1)
                            lst = lst[:cut] + [pend_tp] + lst[cut:]
                        av_units += lst
                        pend_tp = mk_tpose(qcl)
                    av_units.append(pend_tp)
                    po_units = []
                    for oc in range(DC):
                        po_units.append(mk_po(oc))
                    return av_units, po_units

                # software pipeline over positions: attnV/norm of pos-1 and
                # outproj of pos-2 interleave between the scores chunks of pos
                # (proportional merge) so the in-order PE queue always has
                # ready work
                avpo = {}
                for it in range(NQ + 1):
                    units = []
                    if 1 <= it <= NQ:
                        avpo[it - 1] = attn_units(it - 1, ORDER[it - 1])
                        units += avpo[it - 1][0]
                    if it >= 2:
                        units += avpo.pop(it - 2)[1]
                    if it < NQ:
                        qt = ORDER[it]
                        nkc = DIAG * qt + DIAG
                        done = 0
                        for kc in range(nkc):
                            score_unit(scp, it, qt, kc)
                            want = (kc + 1) * len(units) // nkc
                            while done < want:
                                units[done]()
                                done += 1
                        while done < len(units):
                            units[done]()
                            done += 1
                    else:
                        # tail: interleave outproj(NQ-2) into the last attnV
                        # stream; outproj(NQ-1) must be EMITTED after the
                        # norms it reads (emission order defines deps), so it
                        # follows sequentially with qcl-split sub-matmuls
                        a = avpo[it - 1][0]
                        b = units[len(a):]
                        ib = 0
                        n = max(len(a), 1)
                        for i_ in range(len(a)):
                            a[i_]()
                            want = (i_ + 1) * len(b) // n
                            while ib < want:
                                b[ib]()
                                ib += 1
                        while ib < len(b):
                            b[ib]()
                            ib += 1
                        for u in avpo.pop(it - 1)[1]:
                            u()

    nc.compile()
    _BUILD_CACHE[key] = nc
    return nc


def host_prep(x, Wq, Wk, Wv, Wo, S=S_FULL):
    """Build per-core input maps (numpy, fp16)."""
    x = np.asarray(x, np.float32).reshape(S, D)
    xT = np.ascontiguousarray(x.T, dtype=np.float16)

    # RoPE pair layout: per head (64 rows), two 32-row quadrant groups:
    # [e0..e15, o0..o15, e16..e31, o16..o31] so the pair swap is 16-apart
    # within each 32-partition quadrant (stream_shuffle-able).
    e = np.arange(0, 64, 2)
    o = np.arange(1, 64, 2)
    perm64 = np.concatenate([e[0:16], o[0:16], e[16:32], o[16:32]])
    # cos/sin rate index per row of the 64-row block
    pair_idx = np.concatenate(
        [np.arange(0, 16), np.arange(0, 16), np.arange(16, 32), np.arange(16, 32)]
    )
    # e-rows get -sin, o-rows +sin
    sgn64 = np.concatenate(
        [-np.ones(16), np.ones(16), -np.ones(16), np.ones(16)]
    )

    j32 = np.arange(32, dtype=np.float64)
    rates = THETA ** (-2.0 * j32 / DK)
    pos = np.arange(S, dtype=np.float64)
    ang = rates[:, None] * pos[None, :]  # (32, S)
    cos32 = np.cos(ang)
    sin32 = np.sin(ang)
    cosd = np.tile(cos32[pair_idx, :], (2, 1)).astype(np.float16)  # (128, S)
    sind = (np.tile(sin32[pair_idx, :], (2, 1)) * np.tile(sgn64, 2)[:, None]).astype(
        np.float16
    )

    DIAG = QT // KC
    r = np.arange(P)[:, None, None]
    jj = np.arange(DIAG)[None, :, None]
    q_local = np.arange(QT)[None, None, :]
    maskd = np.where(q_local >= jj * KC + r, 0.0, -10000.0).astype(np.float16)

    ident = np.eye(P, dtype=np.float16)

    def swizzle_w(Wslc):
        # [128 rows of W, 1024 cols] -> [p, c, m] with value W.T[c*128+p, m]
        t = np.ascontiguousarray(np.asarray(Wslc).T, dtype=np.float16)  # (1024, 128)
        return t.reshape(DC, P, P).transpose(1, 0, 2)  # (P, DC, P)

    csd = np.stack([cosd, sind], axis=1).reshape(P, 2 * S)

    in_maps = []
    for g in range(N_CORES):
        h0, h1 = 2 * g, 2 * g + 1
        idx_qk = np.concatenate([h0 * DK + perm64, h1 * DK + perm64])
        idx_v = np.arange(h0 * DK, h0 * DK + 2 * DK)
        wqkv = np.stack(
            [
                swizzle_w(np.asarray(Wq)[idx_qk, :]),
                swizzle_w(np.asarray(Wk)[idx_qk, :]),
                swizzle_w(np.asarray(Wv)[idx_v, :]),
            ],
            axis=2,
        )  # (P, DC, 3, P)
        woT = np.asarray(Wo)[:, idx_v].T.astype(np.float16)  # (P, D)
        cstd = np.concatenate(
            [woT, maskd.reshape(P, DIAG * QT), ident], axis=1
        )
        in_maps.append(
            {
                "xT": xT,
                "wqkvR": np.ascontiguousarray(wqkv.reshape(P, DC * 3 * P)),
                "csd": np.ascontiguousarray(csd),
                "cstd": np.ascontiguousarray(cstd),
            }
        )
    return in_maps


def run_cores(x, Wq, Wk, Wv, Wo, S=S_FULL, core_ids=None, trace=False):
    from concourse.bass_utils import run_bass_kernel_spmd

    nc = build(S)
    in_maps = host_prep(x, Wq, Wk, Wv, Wo, S=S)
    if core_ids is None:
        core_ids = list(range(N_CORES))
    in_maps = in_maps[: len(core_ids)]
    res = run_bass_kernel_spmd(nc, in_maps, core_ids, trace=trace)
    return res


def kernel(x, Wq, Wk, Wv, Wo):
    x = np.asarray(x, np.float32)
    res = run_cores(x, np.asarray(Wq), np.asarray(Wk), np.asarray(Wv), np.asarray(Wo))
    y = np.zeros((D, S_FULL), np.float64)
    for r in res.results:
        y += r["yT"].astype(np.float64)
    return np.ascontiguousarray(y.T, dtype=np.float32).reshape(1, S_FULL, D)


# revision 82
# speedup vs baseline: 1.0239x; 1.0239x over previous
"""Multi-head causal self-attention with RoPE on 8 Trainium2 NeuronCores.

Sharding: 16 heads -> 8 cores (2 heads/core, head/tensor parallel).
Wq/Wk/Wv column-sharded (per-head-group rows of W), Wo row-sharded.
Each core computes a full (S, D) partial of the output projection in fp16;
the host sums the 8 partials (the row-parallel reduce).

v3 layout notes (cost-model driven, on top of v2; 230us -> ~204us):
 - q-tiles processed big/small interleaved (7,0,6,1,5,2,4,3): every
   small-tile iteration carries a big attnV stream to fill the in-order PE
   queue, exp demand stays ~balanced against PE supply, and the tail tile
   (qt=3) is big enough to self-overlap its norm/outproj chains.
 - RoPE pair layout regrouped per 32-partition quadrant ([e0..15 o0..15]
   per 32 rows) so the even/odd swap is a single DVE stream_shuffle per
   tile instead of 4 partition-swap DMAs (saves ~630ns HWDGE + ~600ns SEQ
   per DMA); the sin-multiply runs on the otherwise-idle GPSIMD.
 - exp work split Act/DVE by greedy cost balance (~53/47) instead of 75/25;
   DVE chunks use the Schraudolph int16 bit-trick, Act chunks exact exp;
   yrow drains and norm scales ride the same balancer.
 - softmax normalize: one strided reciprocal -> per-head scale into asb ->
   PE transpose into fp16 scratch inside the SAME att psum bank (bitcast
   view; attnV's start=True bank-wipe is ordered by pool rotation) ->
   engine copy to attnT. ~0.7us chain vs ~3us for the old DMA transpose.
 - tail: outproj psums rotate through the then-idle 3-deep score pool, the
   last two positions' outproj matmuls split per q-subblock, and the last
   attnV stream interleaves both remaining outproj streams.
 - DMA instruction count minimized (each costs ~630ns on the shared
   HWDGE): wq|wk|wv pre-swizzled into one DRAM tensor (2 loads), cos|sin
   combined and streamed per-2-tiles behind the x prefetch, wo|mask|ident
   one blob, output staged in oc-pair tiles (3-deep) DMA'd from SP, ones
   column via GPSIMD memset.
"""

import sys

for _p in ("/opt/trn_rl_repo", "/root/.axon_site/_ro/trn_rl_repo"):
    if _p not in sys.path:
        sys.path.insert(0, _p)

import numpy as np

S_FULL = 4096
D = 1024
NH = 16
DK = 64
P = 128
QT = 512  # q tile (free dim of score tiles)
KC = 128  # k chunk (partition dim of score tiles)
DC = D // P  # 8 contraction chunks for the projections
THETA = 10000.0
N_CORES = 8

# Schraudolph exp constants for fp16 bit pattern: round(s*A + B) ~ fp16(exp(s/8))
LOG2E = 1.4426950408889634
EXP_A = (1 << 10) * LOG2E * 0.125
EXP_B = 15.0 * (1 << 10) - 58.9

# engine-cost constants (ns) used for greedy Act/DVE balancing at build time
ACT_NS = 0.8333
DVE_NS = 1.0417
ACT_OV = 185.0
DVE_OV = 125.0

_BUILD_CACHE: dict = {}


class EngBalance:
    """Greedy Act/DVE load balancer for psum-reading elementwise ops.

    act0 pre-loads the Act side to account for work outside the balancer's
    view (SEQ overheads, exact-exp affinity) observed in the trace.
    """

    def __init__(self, act0=0.0):
        self.act = act0
        self.dve = 0.0

    def pick(self, cols, bias_act=0.0):
        act_c = cols * ACT_NS + ACT_OV
        dve_c = cols * DVE_NS + DVE_OV
        if self.act + act_c + bias_act <= self.dve + dve_c:
            self.act += act_c
            return "act"
        self.dve += dve_c
        return "dve"


def build(S: int = S_FULL, reps: int = 1):
    """Build the per-core Bass program (same program for all cores)."""
    key = (S, reps)
    if key in _BUILD_CACHE:
        return _BUILD_CACHE[key]

    import concourse.bacc as bacc
    import concourse.tile as tile
    from concourse import mybir

    f32 = mybir.dt.float32
    f16 = mybir.dt.float16
    i16 = mybir.dt.int16
    Alu = mybir.AluOpType
    Act = mybir.ActivationFunctionType

    NQ = S // QT
    NK = S // KC
    DIAG = QT // KC  # k-chunks per q-tile on the diagonal (4)
    SCALE = float(DK) ** -0.5
    # big/small interleave: every small-tile iteration carries a big attnV
    # stream (and vice versa), the exp pipeline stays ~balanced against PE,
    # and the tail tile (qt=3) is big enough to self-overlap its chains.
    ORDER = []
    lo, hi = 0, NQ - 1
    while lo <= hi:
        ORDER.append(hi)
        if lo < hi:
            ORDER.append(lo)
        hi -= 1
        lo += 1
    ESB_N = max(DIAG * ORDER[p] + DIAG for p in range(1, NQ, 2))
    SHUF_MASK = list(range(16, 32)) + list(range(0, 16))  # swap 16-halves

    nc = bacc.Bacc(
        "TRN2", target_bir_lowering=False, debug=False, num_devices=N_CORES
    )
    xT = nc.dram_tensor("xT", [D, S], f16, kind="ExternalInput")
    # weights pre-swizzled on host to [p, c, w(q|k|v), m] rows so startup is
    # 2 DMA instructions, not 6 (each DMA pays ~630ns on the shared HWDGE)
    wqkvR = nc.dram_tensor("wqkvR", [P, DC * 3 * P], f16, kind="ExternalInput")
    csd = nc.dram_tensor("csd", [P, 2 * S], f16, kind="ExternalInput")
    # wo | mask | ident concatenated: one late-constants DMA
    cstd = nc.dram_tensor("cstd", [P, D + DIAG * QT + P], f16, kind="ExternalInput")
    yT = nc.dram_tensor("yT", [D, S], f16, kind="ExternalOutput")

    bal = EngBalance(act0=0.0)

    with tile.TileContext(nc) as tc:
        with (
            tc.tile_pool(name="const", bufs=1) as cp,
            tc.tile_pool(name="persist", bufs=1) as pp,
        ):
            # ---- constants ----
            wqkv_sb = cp.tile([P, DC, 3, P], f16, tag="wqkv")
            cs_sb = cp.tile([P, 2, S], f16, tag="cs")
            cst_sb = cp.tile([P, D + DIAG * QT + P], f16, tag="cst")
            wo_sb = cst_sb[:, 0:D]
            mask_sb = cst_sb[:, D : D + DIAG * QT].rearrange(
                "p (j q) -> p j q", j=DIAG
            )
            id_sb = cst_sb[:, D + DIAG * QT :]

            # ---- persistent activations ----
            qT_sb = pp.tile([P, S], f16, tag="qT")
            kT_sb = pp.tile([P, S], f16, tag="kT")
            v1a = pp.tile([P, NK, 65], f16, tag="v1a")  # head 0: [v, ones]
            v1b = pp.tile([P, NK, 65], f16, tag="v1b")  # head 1
            # es double-buffered by position parity; even positions are the
            # larger tiles (qt = 7,5,3,1 -> up to NK chunks)
            es_a = pp.tile([P, NK, 2, QT], f16, tag="esa")
            es_b = pp.tile([P, ESB_N, 2, QT], f16, tag="esb")
            attnT = pp.tile([P, 2, QT], f16, tag="attnT")  # parity-buffered

            def es_for(pos):
                return es_a if pos % 2 == 0 else es_b

            def emit_exp(es_f16, es_i16, ps_ap, cols, force_act=False):
                """ps_ap: psum fp32 source; es_*: fp16/int16 SBUF views."""
                if force_act:
                    bal.act += cols * ACT_NS + ACT_OV
                    eng = "act"
                else:
                    eng = bal.pick(cols)
                if eng == "act":
                    nc.scalar.activation(es_f16, ps_ap, Act.Exp, scale=SCALE)
                else:
                    nc.vector.tensor_scalar(
                        out=es_i16,
                        in0=ps_ap,
                        scalar1=float(EXP_A),
                        scalar2=float(EXP_B),
                        op0=Alu.mult,
                        op1=Alu.add,
                    )

            def score_unit(pool, pos, qt, kc):
                q0 = qt * QT
                es_all = es_for(pos)
                j = kc - DIAG * qt  # >=0 on the diagonal
                qoff = j * KC if j >= 0 else 0
                ksl = slice(kc * KC, (kc + 1) * KC)
                ps = pool.tile([P, 2, QT], f32, tag="sc")
                diag = j >= 0
                if diag:
                    # causal mask folded in as a -1e4 bias via an
                    # identity matmul into the psum (per head)
                    for h in range(2):
                        nc.tensor.matmul(
                            ps[:, h, qoff : qoff + KC],
                            id_sb,
                            mask_sb[:, j, qoff : qoff + KC],
                            start=True, stop=False,
                        )
                nc.tensor.matmul(
                    ps[:, 0, qoff:QT],
                    kT_sb[0:64, ksl],
                    qT_sb[0:64, q0 + qoff : q0 + QT],
                    start=not diag, stop=True, tile_position=(0, 0),
                )
                nc.tensor.matmul(
                    ps[:, 1, qoff:QT],
                    kT_sb[64:128, ksl],
                    qT_sb[64:128, q0 + qoff : q0 + QT],
                    start=not diag, stop=True, tile_position=(64, 0),
                )
                if j >= 1:
                    # narrow per-head exp (saves engine cols on the diag);
                    # diag chunks carry the big probs -> exact exp on Act
                    for h in range(2):
                        emit_exp(
                            es_all[:, kc, h, qoff:QT],
                            es_all.bitcast(i16)[:, kc, h, qoff:QT],
                            ps[:, h, qoff:QT],
                            QT - qoff,
                            force_act=True,
                        )
                else:
                    emit_exp(
                        es_all[:, kc, :, :].rearrange("p h q -> p (h q)"),
                        es_all.bitcast(i16)[:, kc, :, :].rearrange(
                            "p h q -> p (h q)"
                        ),
                        ps.rearrange("p h q -> p (h q)"),
                        2 * QT,
                        force_act=(j == 0),
                    )

            # ---- phase A: projections + RoPE + v-transposes, per 512-col
            # chunk; the first phase-E position's (qt = NQ-1) scores + exp
            # are FUSED into this phase: its q/k tile ropes first, and each
            # roped k-tile's 4 score chunks emit one tile-section later,
            # interspersed in the projection c-loop. Their psums rotate in a
            # dedicated 2-buf pool that closes with phase A. ----
            TORD = list(range(NQ))  # tile process order
            with (
                tc.tile_pool(name="xc", bufs=4) as xcp,
                tc.tile_pool(name="rope", bufs=2) as rp,
                tc.tile_pool(name="proj_ps", bufs=2, space="PSUM") as pps,
                tc.tile_pool(name="tp_ps", bufs=2, space="PSUM") as tpp,
            ):
                def fetch_chunk(nt):
                    sl = slice(nt * QT, (nt + 1) * QT)
                    xc = xcp.tile([P, DC, QT], f16, tag="xc")
                    xv = xT[:, sl].rearrange("(c p) q -> p c q", p=P)
                    if nt == TORD[0]:
                        # split so the first projection matmuls start sooner
                        nc.scalar.dma_start(out=xc[:, 0:1, :], in_=xv[:, 0:1, :])
                        nc.scalar.dma_start(out=xc[:, 1:3, :], in_=xv[:, 1:3, :])
                        nc.scalar.dma_start(out=xc[:, 3:DC, :], in_=xv[:, 3:DC, :])
                    else:
                        nc.scalar.dma_start(out=xc, in_=xv)
                    return xc

                # the scalar(Act) DMA queue carries ONLY x chunks; everything
                # else rides the sync(SP) queue ordered by first-use time
                wv3 = wqkvR[:, :].rearrange("p (c w m) -> p c w m", c=DC, w=3)
                csv = csd[:, :].rearrange("p (t s) -> p t s", t=2)
                # c0 chunk of all three weights first (tile0's c-loop
                # interleaves q/k/v matmuls, so all three gate the start)
                nc.sync.dma_start(out=wqkv_sb[:, 0:1, :, :], in_=wv3[:, 0:1, :, :])
                prefs = [fetch_chunk(TORD[0]), fetch_chunk(TORD[1]), fetch_chunk(TORD[2])]
                nc.sync.dma_start(out=wqkv_sb[:, 1:DC, :, :], in_=wv3[:, 1:DC, :, :])
                # cos/sin for the first two tiles in process order; the rest
                # streams per-2-tiles inside the loop so the big transfers
                # don't crowd the x chunks off the DMA pool
                nc.sync.dma_start(out=cs_sb[:, :, 0 : 2 * QT], in_=csv[:, :, 0 : 2 * QT])
                # Z-accumulator ones column via GPSIMD memset (no DMA needed)
                nc.gpsimd.memset(v1a[:, :, 64:65], 1.0)
                nc.gpsimd.memset(v1b[:, :, 64:65], 1.0)

                vt_prev = None
                for idx in range(NQ):
                    nt = TORD[idx]
                    sl = slice(nt * QT, (nt + 1) * QT)
                    xc = prefs.pop(0)
                    if idx + 3 < NQ:
                        prefs.append(fetch_chunk(TORD[idx + 3]))
                    if idx in (1, 3, 5):
                        s0 = nt * QT + QT  # next two tiles in ascending order
                        nc.sync.dma_start(
                            out=cs_sb[:, :, s0 : s0 + 2 * QT],
                            in_=csv[:, :, s0 : s0 + 2 * QT],
                        )
                    if idx == 0:
                        # ident (needed by vtrans(t0) at ~10us) + wo/mask
                        nc.sync.dma_start(out=cst_sb, in_=cstd[:, :])
                    psq = pps.tile([P, QT], f32, tag="psq")
                    psk = pps.tile([P, QT], f32, tag="psk")
                    psv = pps.tile([P, QT], f32, tag="psv")
                    for c in range(DC):
                        st, sp = (c == 0), (c == DC - 1)
                        nc.tensor.matmul(psq, wqkv_sb[:, c, 0, :], xc[:, c, :], start=st, stop=sp)
                        nc.tensor.matmul(psk, wqkv_sb[:, c, 1, :], xc[:, c, :], start=st, stop=sp)
                        nc.tensor.matmul(psv, wqkv_sb[:, c, 2, :], xc[:, c, :], start=st, stop=sp)
                    vt = rp.tile([P, QT], f16, tag="vt")
                    nc.scalar.copy(qT_sb[:, sl], psq)
                    nc.scalar.copy(kT_sb[:, sl], psk)
                    nc.scalar.copy(vt, psv)
                    # RoPE on this chunk (in place): the even/odd swap is one
                    # DVE stream_shuffle (pairs are 16 apart within each
                    # 32-partition quadrant); sin-mul runs on GPSIMD
                    for src_sb, tgname in ((qT_sb, "swq"), (kT_sb, "swk")):
                        sw = rp.tile([P, QT], f16, tag=tgname)
                        nc.vector.stream_shuffle(sw, src_sb[:, sl], SHUF_MASK)
                        m1 = rp.tile([P, QT], f16, tag="m1", name="m1")
                        nc.vector.tensor_mul(m1, src_sb[:, sl], cs_sb[:, 0, sl])
                        nc.gpsimd.tensor_mul(sw, sw, cs_sb[:, 1, sl])
                        nc.vector.tensor_add(src_sb[:, sl], m1, sw)

                    # v~ transposes lag one chunk so PE never waits on the
                    # fresh vt copy
                    def vtrans(nt, vt):
                        for h, v1 in ((0, v1a), (1, v1b)):
                            hp = h * 64
                            pst = tpp.tile([P, DIAG, 64], f16, tag="pst", name="pst")
                            for j in range(DIAG):
                                nc.tensor.transpose(
                                    pst[:, j, :],
                                    vt[hp : hp + 64, j * KC : (j + 1) * KC],
                                    id_sb[hp : hp + 64, hp : hp + 64],
                                )
                            nc.scalar.copy(
                                v1[:, DIAG * nt : DIAG * nt + DIAG, 0:64], pst
                            )
                    if vt_prev is not None:
                        vtrans(*vt_prev)
                    vt_prev = (nt, vt)
                if vt_prev is not None:
                    vtrans(*vt_prev)

            # ---- phase E: scores+exp, attnV, outproj, per q tile ----
            with (
                tc.tile_pool(name="sc_ps", bufs=3, space="PSUM") as scp,
                tc.tile_pool(name="att_ps", bufs=1, space="PSUM") as attp,
                tc.tile_pool(name="po_ps", bufs=1, space="PSUM") as pop,
                tc.tile_pool(name="asb", bufs=4) as asp,
                tc.tile_pool(name="yrow", bufs=3) as yrp,
            ):
                def attn_units(pos, qt):
                    """Closures for attnV matmul steps, norms, and outproj of
                    q-tile qt; interleaved between scores chunks of the next
                    position so the in-order PE queue always has ready work."""
                    es_all = es_for(pos)
                    par = pos % 2
                    q0 = qt * QT
                    state = {}

                    def pa_slot(qcl, h):
                        # per head: [h*66 : h*66+66] = [attn 65 | Z 1]; f16
                        # cols [264:392] of the tile (same bank) are the
                        # PE-transpose scratch. attnV start=True zeroes the
                        # whole bank, so each qcl gets its OWN tile; pool
                        # rotation orders the bank wipe after the previous
                        # qcl's transpose copy.
                        return state[qcl][:, h * 66 : h * 66 + 66]

                    def mk_av(qcl, kc, qc):
                        def f():
                            if kc == 0:
                                if pos >= NQ - 2:
                                    # tail: rotate through the idle 3-deep
                                    # score pool so qcl chains overlap
                                    t = scp.tile(
                                        [P, 2, QT], f32, tag="sc", name="pa"
                                    )
                                    state[qcl] = t[:, 0, 0:196]
                                    state["tp%d" % qcl] = t.bitcast(f16)[
                                        :, 0, 264:392
                                    ]
                                elif pos == 0 and qcl % 2:
                                    # first position has no concurrent
                                    # outproj -> borrow the idle po bank so
                                    # the big attnV's qcl chains double-buffer
                                    t = pop.tile([P, QT], f32, tag="po", name="pa")
                                    state[qcl] = t[:, 0:196]
                                    state["tp%d" % qcl] = t.bitcast(f16)[:, 264:392]
                                else:
                                    t = attp.tile(
                                        [P, 196], f32, tag="att", name="pa"
                                    )
                                    state[qcl] = t
                                    state["tp%d" % qcl] = t.bitcast(f16)[:, 264:392]
                            nc.tensor.matmul(
                                pa_slot(qcl, 0)[:, 0:65],
                                es_all[:, kc, 0, qcl * KC : (qcl + 1) * KC],
                                v1a[:, kc, :],
                                start=(kc == 0), stop=False,
                            )
                            nc.tensor.matmul(
                                pa_slot(qcl, 1)[:, 0:65],
                                es_all[:, kc, 1, qcl * KC : (qcl + 1) * KC],
                                v1b[:, kc, :],
                                start=False, stop=(kc == qc),
                            )
                        return f

                    def mk_norm(qcl):
                        def f():
                            zrec = asp.tile([P, 2, 1], f32, tag="zrec", name="zrec")
                            asb = asp.tile([P, P], f16, tag="asb", name="asb")
                            zs = state[qcl][:, 0:132].rearrange(
                                "p (h c) -> p h c", h=2
                            )[:, :, 64:65]
                            nc.vector.reciprocal(zrec, zs)
                            bal.dve += 2 * DVE_NS + DVE_OV
                            for h in range(2):
                                if bal.pick(64) == "act":
                                    nc.scalar.activation(
                                        asb[:, h * 64 : (h + 1) * 64],
                                        pa_slot(qcl, h)[:, 0:64],
                                        Act.Copy,
                                        scale=zrec[:, h, :],
                                    )
                                else:
                                    nc.vector.tensor_scalar(
                                        out=asb[:, h * 64 : (h + 1) * 64],
                                        in0=pa_slot(qcl, h)[:, 0:64],
                                        scalar1=zrec[:, h, :],
                                        scalar2=None,
                                        op0=Alu.mult,
                                    )
                            state["asb%d" % qcl] = asb
                        return f

                    def mk_tpose(qcl):
                        def f():
                            asb = state.pop("asb%d" % qcl)
                            tp = state.pop("tp%d" % qcl)
                            nc.tensor.transpose(tp, asb, id_sb)
                            if bal.pick(KC) == "act":
                                nc.scalar.copy(
                                    attnT[:, par, qcl * KC : (qcl + 1) * KC], tp
                                )
                            else:
                                nc.vector.tensor_copy(
                                    attnT[:, par, qcl * KC : (qcl + 1) * KC], tp
                                )
                        return f

                    def mk_po(oc):
                        def f():
                            if oc % 2 == 0:
                                # oc-pair staging tiles, 3-deep rotation, so a
                                # drain never waits on the previous position's
                                # full output DMA
                                state["yr"] = yrp.tile([P, 2, QT], f16, tag="yrow", name="yrow")
                            if pos >= NQ - 2:
                                # tail: scores are done, so rotate outproj
                                # psums through the 3-deep score pool to
                                # break the single-buf drain->matmul chain
                                po = scp.tile([P, 2, QT], f32, tag="sc", name="po")[:, 0, :]
                            else:
                                po = pop.tile([P, QT], f32, tag="po", name="po")
                            if pos >= NQ - 2:
                                # tail: split by q-subblock so each sub-matmul
                                # only waits on its own qcl's DMA-transpose
                                for qcl in range(DIAG):
                                    qs = slice(qcl * KC, (qcl + 1) * KC)
                                    nc.tensor.matmul(
                                        po[:, qs],
                                        wo_sb[:, oc * P : (oc + 1) * P],
                                        attnT[:, par, qs],
                                        start=True, stop=True,
                                    )
                            else:
                                nc.tensor.matmul(
                                    po, wo_sb[:, oc * P : (oc + 1) * P],
                                    attnT[:, par, :],
                                    start=True, stop=True,
                                )
                            if bal.pick(QT) == "act":
                                nc.scalar.copy(state["yr"][:, oc % 2, :], po)
                            else:
                                nc.vector.tensor_copy(state["yr"][:, oc % 2, :], po)
                            qsl = slice(q0, q0 + QT)
                            yv = yT[:, qsl].rearrange("(c p) q -> p c q", p=P)
                            if oc % 2 == 1:
                                nc.sync.dma_start(
                                    out=yv[:, oc - 1 : oc + 1, :], in_=state["yr"]
                                )
                        return f

                    # weave: each qcl's PE-transpose unit is delayed 2 slots
                    # into the next qcl's stream so it doesn't park the
                    # in-order PE queue while waiting on the asb scales
                    av_units = []
                    pend_tp = None
                    for qcl in range(DIAG):
                        lst = []
                        qc = DIAG * qt + qcl
                        for kc in range(qc + 1):
                            lst.append(mk_av(qcl, kc, qc))
                        lst.append(mk_norm(qcl))
                        if pend_tp is not None:
                            cut = min(2, len(lst) - 1)
                            lst = lst[:cut] + [pend_tp] + lst[cut:]
                        av_units += lst
                        pend_tp = mk_tpose(qcl)
                    av_units.append(pend_tp)
                    po_units = []
                    for oc in range(DC):
                        po_units.append(mk_po(oc))
                    return av_units, po_units

                # software pipeline over positions: attnV/norm of pos-1 and
                # outproj of pos-2 interleave between the scores chunks of pos
                # (proportional merge) so the in-order PE queue always has
                # ready work
                avpo = {}
                for it in range(NQ + 1):
                    units = []
                    if 1 <= it <= NQ:
                        avpo[it - 1] = attn_units(it - 1, ORDER[it - 1])
                        units += avpo[it - 1][0]
                    if it >= 2:
                        units += avpo.pop(it - 2)[1]
                    if it < NQ:
                        qt = ORDER[it]
                        nkc = DIAG * qt + DIAG
                        done = 0
                        for kc in range(nkc):
                            score_unit(scp, it, qt, kc)
                            want = (kc + 1) * len(units) // nkc
                            while done < want:
                                units[done]()
                                done += 1
                        while done < len(units):
                            units[done]()
                            done += 1
                    else:
                        # tail: interleave outproj(NQ-2) into the last attnV
                        # stream; outproj(NQ-1) must be EMITTED after the
                        # norms it reads (emission order defines deps), so it
                        # follows sequentially with qcl-split sub-matmuls
                        a = avpo[it - 1][0]
                        b = units[len(a):]
                        ib = 0
                        n = max(len(a), 1)
                        for i_ in range(len(a)):
                            a[i_]()
                            want = (i_ + 1) * len(b) // n
                            while ib < want:
                                b[ib]()
                                ib += 1
                        while ib < len(b):
                            b[ib]()
                            ib += 1
                        for u in avpo.pop(it - 1)[1]:
                            u()

    nc.compile()
    _BUILD_CACHE[key] = nc
    return nc


def host_prep(x, Wq, Wk, Wv, Wo, S=S_FULL):
    """Build per-core input maps (numpy, fp16)."""
    x = np.asarray(x, np.float32).reshape(S, D)
    xT = np.ascontiguousarray(x.T, dtype=np.float16)

    # RoPE pair layout: per head (64 rows), two 32-row quadrant groups:
    # [e0..e15, o0..o15, e16..e31, o16..o31] so the pair swap is 16-apart
    # within each 32-partition quadrant (stream_shuffle-able).
    e = np.arange(0, 64, 2)
    o = np.arange(1, 64, 2)
    perm64 = np.concatenate([e[0:16], o[0:16], e[16:32], o[16:32]])
    # cos/sin rate index per row of the 64-row block
    pair_idx = np.concatenate(
        [np.arange(0, 16), np.arange(0, 16), np.arange(16, 32), np.arange(16, 32)]
    )
    # e-rows get -sin, o-rows +sin
    sgn64 = np.concatenate(
        [-np.ones(16), np.ones(16), -np.ones(16), np.ones(16)]
    )

    j32 = np.arange(32, dtype=np.float64)
    rates = THETA ** (-2.0 * j32 / DK)
    pos = np.arange(S, dtype=np.float64)
    ang = rates[:, None] * pos[None, :]  # (32, S)
    cos32 = np.cos(ang)
    sin32 = np.sin(ang)
    cosd = np.tile(cos32[pair_idx, :], (2, 1)).astype(np.float16)  # (128, S)
    sind = (np.tile(sin32[pair_idx, :], (2, 1)) * np.tile(sgn64, 2)[:, None]).astype(
        np.float16
    )

    DIAG = QT // KC
    r = np.arange(P)[:, None, None]
    jj = np.arange(DIAG)[None, :, None]
    q_local = np.arange(QT)[None, None, :]
    maskd = np.where(q_local >= jj * KC + r, 0.0, -10000.0).astype(np.float16)

    ident = np.eye(P, dtype=np.float16)

    def swizzle_w(Wslc):
        # [128 rows of W, 1024 cols] -> [p, c, m] with value W.T[c*128+p, m]
        t = np.ascontiguousarray(np.asarray(Wslc).T, dtype=np.float16)  # (1024, 128)
        return t.reshape(DC, P, P).transpose(1, 0, 2)  # (P, DC, P)

    csd = np.stack([cosd, sind], axis=1).reshape(P, 2 * S)

    in_maps = []
    for g in range(N_CORES):
        h0, h1 = 2 * g, 2 * g + 1
        idx_qk = np.concatenate([h0 * DK + perm64, h1 * DK + perm64])
        idx_v = np.arange(h0 * DK, h0 * DK + 2 * DK)
        wqkv = np.stack(
            [
                swizzle_w(np.asarray(Wq)[idx_qk, :]),
                swizzle_w(np.asarray(Wk)[idx_qk, :]),
                swizzle_w(np.asarray(Wv)[idx_v, :]),
            ],
            axis=2,
        )  # (P, DC, 3, P)
        woT = np.asarray(Wo)[:, idx_v].T.astype(np.float16)  # (P, D)
        cstd = np.concatenate(
            [woT, maskd.reshape(P, DIAG * QT), ident], axis=1
        )
        in_maps.append(
            {
                "xT": xT,
                "wqkvR": np.ascontiguousarray(wqkv.reshape(P, DC * 3 * P)),
                "csd": np.ascontiguousarray(csd),
                "cstd": np.ascontiguousarray(cstd),
            }
        )
    return in_maps


def run_cores(x, Wq, Wk, Wv, Wo, S=S_FULL, core_ids=None, trace=False):
    from concourse.bass_utils import run_bass_kernel_spmd

    nc = build(S)
    in_maps = host_prep(x, Wq, Wk, Wv, Wo, S=S)
    if core_ids is None:
        core_ids = list(range(N_CORES))
    in_maps = in_maps[: len(core_ids)]
    res = run_bass_kernel_spmd(nc, in_maps, core_ids, trace=trace)
    return res


def kernel(x, Wq, Wk, Wv, Wo):
    x = np.asarray(x, np.float32)
    res = run_cores(x, np.asarray(Wq), np.asarray(Wk), np.asarray(Wv), np.asarray(Wo))
    y = np.zeros((D, S_FULL), np.float64)
    for r in res.results:
        y += r["yT"].astype(np.float64)
    return np.ascontiguousarray(y.T, dtype=np.float32).reshape(1, S_FULL, D)
